# revision 57
# baseline (speedup 1.0000x reference)
"""Trainium2 Bass kernel for nn_AdditiveAttention (B=8, S=4096, D=1024, H=16).

Sharding: pure data-parallel over batch — 8 NeuronCores, one batch element
per core, weights replicated. No collectives.

v2 layout (everything transposed: d on partitions, s on free):
  - Q GEMM is n-outer (s-chunk outer, t-tile inner) so each xt s-chunk is
    dead right after its 8 output tiles are computed; q (bf16, +bq+br folded
    in) is written back into the xt chunk buffers with a one-chunk lag.
    Only one spare [128, 8, 512] buffer is needed for q chunk 0.
  - Per-chunk softmax pooling: logit matmul emitted one GEMM-slot late so
    PSUM evacuation always overlaps the next chunk's matmuls; exp+denominator
    fused on ScalarE (accum_out); numerator via one fused DVE
    tensor_tensor_reduce per chunk. No full-S e/p/u tiles anywhere.
  - K/V GEMMs in fp8 DoubleRow (weights host-scaled by 64; the 1/64 rides in
    the pooled-vector normalization), per-chunk gated logits / rt matmuls
    pipelined the same way.
  - Single bf16 output: out = q + (u @ Wr) (+bq+br already inside q),
    residual added during rt-PSUM evacuation on DVE. 8.4 MB written instead
    of the baseline's 33.6 MB f32 out+out2 pair.
  - wk/wv/xt8 prefetched on the scalar queue during the Q phase; startup
    loads are chunk-granular ([128,512]) and ordered so chunk 0 + wq arrive
    first on two issuing queues.
"""

import sys
import types

import numpy as np
import ml_dtypes

from contextlib import ExitStack

import concourse.bass as bass
import concourse.tile as tile
from concourse import bacc, mybir
from concourse.bass_utils import run_bass_kernel_spmd

B, S, D, H, HD = 8, 4096, 1024, 16, 64
P = 128          # partitions
T = D // P       # 8 d-tiles
NC_ = 512        # psum chunk free size
NS = S // NC_    # 8 s-chunks
N_CORES = 8
BF16 = mybir.dt.bfloat16
FP8 = mybir.dt.float8e4
F32 = mybir.dt.float32
W8SCALE = 64.0   # host scales Wk/Wv by this into e4m3 normal range
INV_W8 = 1.0 / W8SCALE
BF = ml_dtypes.bfloat16
F8 = ml_dtypes.float8_e4m3
OUT_DT = BF16  # bf16 halves output DMA traffic; host upcasts
# Pooled-sum (softmax numerator) implementation:
#   'stt_gpsimd': fused (e*(1/64))*src + accum via scalar_tensor_tensor on GpSimd
#   'stt_vector': same fused op on VectorE
#   'split':     tensor_tensor mult + reduce_sum, both on VectorE
POOL_MODE = "stt_vector"

_CACHE = {}


def _build():
    nc = bacc.Bacc(
        "TRN2", target_bir_lowering=False, debug=False, num_devices=N_CORES
    )
    xt_ext = nc.declare_dram_parameter("xt", [D, S], BF16, isOutput=False)
    xt8_ext = nc.declare_dram_parameter("xt8", [D, S], FP8, isOutput=False)
    # wq arrives pre-packed by output-tile column blocks: row p holds
    # [t][k][j] so block t (the stationaries for output tile t, all k) is a
    # contiguous 256KB slab — loadable incrementally in t order
    wq_ext = nc.declare_dram_parameter("wq", [P, T * T * P], BF16, isOutput=False)
    wk_ext = nc.declare_dram_parameter("wk", [D, D], FP8, isOutput=False)
    wv_ext = nc.declare_dram_parameter("wv", [D, D], FP8, isOutput=False)
    bqbr_ext = nc.declare_dram_parameter("bqbr", [P, T], F32, isOutput=False)
    bk_ext = nc.declare_dram_parameter("bk64", [P, T], F32, isOutput=False)
    bv_ext = nc.declare_dram_parameter("bv64", [P, T], F32, isOutput=False)
    br64_ext = nc.declare_dram_parameter("br64", [P, 1], F32, isOutput=False)
    wql_ext = nc.declare_dram_parameter("wqlrep", [P, P], BF16, isOutput=False)
    wkl_ext = nc.declare_dram_parameter("wklrep", [P, P], BF16, isOutput=False)
    wrr_ext = nc.declare_dram_parameter("wrr", [P, P], BF16, isOutput=False)
    out_ext = nc.declare_dram_parameter("out", [D, S], OUT_DT, isOutput=True)

    AX = mybir.AxisListType.X
    ALU = mybir.AluOpType
    AF = mybir.ActivationFunctionType
    DR = mybir.MatmulPerfMode.DoubleRow

    with tile.TileContext(nc) as tc, ExitStack() as ctx:
        singles = ctx.enter_context(tc.tile_pool(name="singles", bufs=1))
        psg = ctx.enter_context(tc.tile_pool(name="psg", bufs=2, space="PSUM"))
        psl = ctx.enter_context(tc.tile_pool(name="psl", bufs=2, space="PSUM"))
        chk_pool = ctx.enter_context(tc.tile_pool(name="chk", bufs=4))
        e_pool = ctx.enter_context(tc.tile_pool(name="epool", bufs=2))
        m_pool = ctx.enter_context(tc.tile_pool(name="mpool", bufs=2))
        # stg depth decouples the V-phase residual adds (and through PSUM
        # WAR, the rt matmuls) from out-DMA completion pacing
        stg_pool = ctx.enter_context(tc.tile_pool(name="stg", bufs=8))
        eff_pool = ctx.enter_context(tc.tile_pool(name="eff", bufs=2))
        small_pool = ctx.enter_context(tc.tile_pool(name="small", bufs=2))

        # ---- resident tiles ----
        xtc = [
            singles.tile([P, T, NC_], BF16, name=f"xtc{n}", tag=f"xtc{n}")
            for n in range(NS)
        ]
        qsp = singles.tile([P, T, NC_], BF16, name="qsp", tag="qsp")
        xt8 = singles.tile([P, T, S], FP8, name="xt8", tag="xt8")
        wq = singles.tile([P, T, T, P], BF16, name="wq", tag="wq")
        wk = singles.tile([P, T, D], FP8, name="wk", tag="wk")
        wv = singles.tile([P, T, D], FP8, name="wv", tag="wv")
        wqlrep = singles.tile([P, P], BF16, name="wqlrep", tag="wqlrep")
        wklrep = singles.tile([P, P], BF16, name="wklrep", tag="wklrep")
        wrr = singles.tile([P, P], BF16, name="wrr", tag="wrr")
        bqbr = singles.tile([P, T], F32, name="bqbr", tag="bqbr")
        bk64 = singles.tile([P, T], F32, name="bk64", tag="bk64")
        bv64 = singles.tile([P, T], F32, name="bv64", tag="bv64")
        br64 = singles.tile([P, 1], F32, name="br64", tag="br64")
        gq_all = singles.tile([P, T], F32, name="gq", tag="gq")
        gk_all = singles.tile([P, T], F32, name="gk", tag="gk")
        pe_q = singles.tile([P, T * NS], F32, name="peq", tag="peq")
        pq_q = singles.tile([P, T * NS], F32, name="pqq", tag="pqq")
        pe_k = singles.tile([P, T * (NS // 2)], F32, name="pek", tag="pek")
        pq_k = singles.tile([P, T * (NS // 2)], F32, name="pqk", tag="pqk")

        pace_sem = nc.alloc_semaphore("pace_sem")

        # ---- DMA issue (ordering matters for startup) ----
        # Chunk 0 goes first, split across both queues, so the first GEMM's
        # data is in front of everything; then wq (alternating), then the
        # remaining chunks with each queue carrying half of every chunk so
        # both queues advance in lockstep with TensorE's chunk consumption.
        def xtc_dma(eng, n, k):
            eng.dma_start(
                xtc[n][:, k, :],
                xt_ext.ap()[k * P : (k + 1) * P, n * NC_ : (n + 1) * NC_],
            )

        def xtc_dma_half(eng, n, k, h):
            hw = NC_ // 2
            eng.dma_start(
                xtc[n][:, k, h * hw : (h + 1) * hw],
                xt_ext.ap()[
                    k * P : (k + 1) * P,
                    n * NC_ + h * hw : n * NC_ + (h + 1) * hw,
                ],
            )

        # wq t-blocks stream in t order, interleaved with the first two xt
        # chunks so supply tracks the Q loop's (chunk, t)-sweep demand.
        # Early transfers are 64KB quarters: one DMA occupies one ring
        # (~22GB/s), so parallelism across rings comes from DMA count.
        QB = T * P // 4  # quarter-block columns

        def wq_dma(t, fine=False):
            base = t * T * P
            if fine:
                for qtr in range(4):
                    eng = nc.sync if qtr < 2 else nc.gpsimd
                    eng.dma_start(
                        wq[:, t, qtr * 2 : qtr * 2 + 2, :],
                        wq_ext.ap()[:, base + qtr * QB : base + (qtr + 1) * QB],
                    )
            else:
                nc.sync.dma_start(
                    wq[:, t, : T // 2, :], wq_ext.ap()[:, base : base + 2 * QB]
                )
                nc.gpsimd.dma_start(
                    wq[:, t, T // 2 :, :],
                    wq_ext.ap()[:, base + 2 * QB : base + T * P],
                )

        wq_dma(0, fine=True)
        for k in range(T):
            xtc_dma(nc.sync if k < T // 2 else nc.gpsimd, 0, k)
        for t in (1, 2, 3):
            wq_dma(t)
        for k in range(3):
            xtc_dma(nc.sync, 1, k)
        for k in range(3, 6):
            xtc_dma(nc.gpsimd, 1, k)
        for t in range(4, T):
            wq_dma(t)
        # scalar: small weights/biases, then K/V weights + fp8 X (all needed
        # only from the K phase on).
        nc.scalar.dma_start(wqlrep[:], wql_ext.ap())
        nc.scalar.dma_start(bqbr[:], bqbr_ext.ap())
        nc.scalar.dma_start(br64[:], br64_ext.ap())
        nc.scalar.dma_start(wklrep[:], wkl_ext.ap())
        nc.scalar.dma_start(wrr[:], wrr_ext.ap())
        nc.scalar.dma_start(bk64[:], bk_ext.ap())
        nc.scalar.dma_start(bv64[:], bv_ext.ap())
        # chunk 1's last slices on the scalar queue (issued immediately)
        for k in (6, 7):
            xtc_dma(nc.scalar, 1, k)
        # Chunks 2..7 are PACED: the DMA rings serve queued descriptors
        # round-robin, so issuing everything up front makes every chunk
        # finish "evenly late". Gate chunk m's issue on chunk m-2's compute
        # (scalar bumps pace_sem at each Q chunk boundary) so at most ~2
        # chunks of loads compete for the rings at once.
        for n in range(2, NS):
            nc.sync.wait_ge(pace_sem, n - 2)
            for k in range(3):
                xtc_dma(nc.sync, n, k)
            nc.gpsimd.wait_ge(pace_sem, n - 2)
            for k in range(3, 6):
                xtc_dma(nc.gpsimd, n, k)

        def kv_prefetch(n):
            """K/V-phase loads, paced: issued on the scalar queue at chunk-n
            boundaries of the Q loop so they don't contend with the Q-phase
            chunk streaming that feeds TensorE."""
            if n == 1:
                for k in range(T):
                    nc.scalar.dma_start(
                        wk[:, k, :], wk_ext.ap()[k * P : (k + 1) * P, :]
                    )
            elif n == 2:
                for k in range(T):
                    nc.scalar.dma_start(
                        wv[:, k, :], wv_ext.ap()[k * P : (k + 1) * P, :]
                    )
            elif 3 <= n <= 6:
                for k in (2 * (n - 3), 2 * (n - 3) + 1):
                    rsl = slice(k * P, (k + 1) * P)
                    nc.scalar.dma_start(
                        xt8[:, k, : S // 2], xt8_ext.ap()[rsl, : S // 2]
                    )
                    nc.scalar.dma_start(
                        xt8[:, k, S // 2 :], xt8_ext.ap()[rsl, S // 2 :]
                    )

        def qreg_of(t, n):
            """q chunk n of tile t lives in xt chunk n-1's space (spare for
            n=0)."""
            src = qsp if n == 0 else xtc[n - 1]
            return src[:, t, :]

        def pool_sum(e, src, accum_col):
            """accum_col = sum_s e[:,s]*src[:,s] / 64 (the 1/64 un-scales the
            fp8 K/V weight scaling; for Q it cancels in num/denom)."""
            m = m_pool.tile(list(e.shape), BF16, name="m", tag="m")
            if POOL_MODE == "stt_vector":
                nc.vector.scalar_tensor_tensor(
                    m, e, INV_W8, src, op0=ALU.mult, op1=ALU.mult,
                    accum_out=accum_col,
                )
            else:
                nc.vector.tensor_tensor(m, e, src, ALU.mult)
                nc.vector.reduce_sum(accum_col, m, axis=AX)

        # ---- Q phase: n-outer so xt chunks free up for q storage ----
        # GEMM and logit PSUM tiles are 2-bank pairs (halves used per slot)
        # so psg+psl fit the 8 PSUM banks alongside the K/V phases' paired
        # layout.
        qpl_state = {}

        def q_tail(t, n):
            """Delayed-by-one-slot logit matmul + exp + pooled partials."""
            qreg = qreg_of(t, n)
            slot = n * T + t
            if slot % 2 == 0:
                qpl_state["pl"] = psl.tile([P, 2, NC_], F32, name="pl", tag="pl")
            pl = qpl_state["pl"][:, slot % 2, :]
            nc.tensor.matmul(pl, wqlrep[:], qreg, start=True, stop=True)
            col = slice(t * NS + n, t * NS + n + 1)
            e = e_pool.tile([P, NC_], BF16, name="e", tag="e")
            nc.scalar.activation(
                e, pl, AF.Exp, bias=0.0, scale=1.0, accum_out=pe_q[:, col]
            )
            pool_sum(e, qreg, pq_q[:, col])

        pend = None
        qpg = None
        for n in range(NS):
            for t in range(T):
                slot = n * T + t
                if slot % 2 == 0:
                    qpg = psg.tile([P, 2, NC_], F32, name="pg", tag="pg")
                pch = qpg[:, slot % 2, :]
                for k in range(T):
                    nc.tensor.matmul(
                        pch, wq[:, t, k, :], xtc[n][:, k, :],
                        start=(k == 0), stop=(k == T - 1),
                    )
                qreg = qreg_of(t, n)
                if slot % 2 == 0:
                    nc.scalar.activation(
                        qreg, pch, AF.Identity, bias=bqbr[:, t : t + 1], scale=1.0
                    )
                else:
                    nc.vector.tensor_scalar_add(qreg, pch, bqbr[:, t : t + 1])
                if pend is not None:
                    q_tail(*pend)
                pend = (t, n)
            # scalar reaches these after chunk n's evac/exp work: paced issue
            nc.scalar.sem_inc(pace_sem, 1)
            if n + 2 < NS:
                for k in (6, 7):
                    xtc_dma(nc.scalar, n + 2, k)
            kv_prefetch(n)
        q_tail(*pend)

        # Q pool finalizers: gq_all = gq_true/64 (br contribution removed).
        for t in range(T):
            tsl = slice(t * NS, (t + 1) * NS)
            stot = small_pool.tile([P, 1], F32, name="stot", tag="stot")
            nc.vector.reduce_sum(stot, pe_q[:, tsl], axis=AX)
            rec = small_pool.tile([P, 1], F32, name="rec", tag="rec")
            nc.vector.reciprocal(rec, stot)
            if POOL_MODE == "split":
                nc.vector.tensor_scalar_mul(rec, rec, INV_W8)
            graw = small_pool.tile([P, 1], F32, name="graw", tag="graw")
            nc.vector.reduce_sum(graw, pq_q[:, tsl], axis=AX)
            tmp = small_pool.tile([P, 1], F32, name="gtmp", tag="gtmp")
            nc.vector.tensor_tensor(tmp, graw, rec, ALU.mult)
            nc.vector.tensor_tensor(gq_all[:, t : t + 1], tmp, br64[:], ALU.subtract)

        # ---- K phase: t-outer, chunk-PAIR pipelined gated logits ----
        # One 2-bank GEMM psum pair per two s-chunks; a single evacuation,
        # exp+denominator, and fused pooled-sum op each cover the whole pair,
        # halving DVE/ACT instruction counts so both stay under TensorE's
        # 1080ns/slot fp8 pace.
        NP2 = NS // 2

        def k_tail(t, np_, pt, eff):
            pl = psl.tile([P, 2, NC_], F32, name="pl", tag="pl")
            nc.tensor.matmul(pl[:, 0, :], eff[:], pt[:, 0, :], start=True, stop=True)
            nc.tensor.matmul(pl[:, 1, :], eff[:], pt[:, 1, :], start=True, stop=True)
            col = slice(t * NP2 + np_, t * NP2 + np_ + 1)
            e = e_pool.tile([P, 2, NC_], BF16, name="e", tag="e")
            nc.scalar.activation(
                e, pl, AF.Exp, bias=0.0, scale=1.0, accum_out=pe_k[:, col]
            )
            pool_sum(e, pt, pq_k[:, col])

        def k_final(t):
            tsl = slice(t * NP2, (t + 1) * NP2)
            stot = small_pool.tile([P, 1], F32, name="stot", tag="stot")
            nc.vector.reduce_sum(stot, pe_k[:, tsl], axis=AX)
            rec = small_pool.tile([P, 1], F32, name="rec", tag="rec")
            nc.vector.reciprocal(rec, stot)
            if POOL_MODE == "split":
                nc.vector.tensor_scalar_mul(rec, rec, INV_W8)
            graw = small_pool.tile([P, 1], F32, name="graw", tag="graw")
            nc.vector.reduce_sum(graw, pq_k[:, tsl], axis=AX)
            tmp = small_pool.tile([P, 1], F32, name="gtmp", tag="gtmp")
            nc.vector.tensor_tensor(tmp, graw, rec, ALU.mult)
            nc.vector.tensor_tensor(
                gk_all[:, t : t + 1], tmp, gq_all[:, t : t + 1], ALU.mult
            )

        kpend = None
        for t in range(T):
            eff = eff_pool.tile([P, P], BF16, name="effkl", tag="eff")
            nc.vector.tensor_scalar_mul(eff[:], wklrep[:], gq_all[:, t : t + 1])
            for np_ in range(NP2):
                pg = psg.tile([P, 2, NC_], F32, name="pg", tag="pg")
                for i in (0, 1):
                    n = 2 * np_ + i
                    for kk in range(0, T, 2):
                        nc.tensor.matmul(
                            pg[:, i, :],
                            wk[:, kk : kk + 2, t * P : (t + 1) * P],
                            xt8[:, kk : kk + 2, n * NC_ : (n + 1) * NC_],
                            start=(kk == 0), stop=(kk == T - 2), perf_mode=DR,
                        )
                pt = chk_pool.tile([P, 2, NC_], BF16, name="chk", tag="chk")
                if np_ % 2 == 0:
                    nc.scalar.activation(
                        pt, pg, AF.Identity, bias=bk64[:, t : t + 1], scale=1.0
                    )
                else:
                    nc.vector.tensor_scalar_add(pt, pg, bk64[:, t : t + 1])
                if kpend is not None:
                    k_tail(*kpend)
                    if kpend[1] == NP2 - 1:
                        k_final(kpend[0])
                kpend = (t, np_, pt, eff)
        k_tail(*kpend)
        k_final(T - 1)

        # ---- V phase: chunk-pair rt matmuls + residual add + store ----
        def v_tail(t, np_, ut, eff):
            pl = psl.tile([P, 2, NC_], F32, name="pl", tag="pl")
            for i in (0, 1):
                n = 2 * np_ + i
                nc.tensor.matmul(
                    pl[:, i, :], eff[:], ut[:, i, :], start=True, stop=True
                )
                stg = stg_pool.tile([P, NC_], OUT_DT, name="stg", tag="stg")
                nc.vector.tensor_tensor(stg, pl[:, i, :], qreg_of(t, n), ALU.add)
                osl = slice(t * P, (t + 1) * P)
                csl = slice(n * NC_, (n + 1) * NC_)
                if t < T - 1:
                    dma_eng = nc.sync if n % 2 == 0 else nc.gpsimd
                    dma_eng.dma_start(out_ext.ap()[osl, csl], stg)
                else:
                    # final tile: split each store across the two DMA queues
                    # that have no compute left so the last transfers drain
                    # on multiple rings
                    h = NC_ // 2
                    nc.sync.dma_start(
                        out_ext.ap()[osl, n * NC_ : n * NC_ + h], stg[:, :h]
                    )
                    nc.gpsimd.dma_start(
                        out_ext.ap()[osl, n * NC_ + h : (n + 1) * NC_], stg[:, h:]
                    )

        vpend = None
        for t in range(T):
            eff = eff_pool.tile([P, P], BF16, name="effrt", tag="eff")
            nc.vector.tensor_scalar_mul(eff[:], wrr[:], gk_all[:, t : t + 1])
            for np_ in range(NP2):
                pg = psg.tile([P, 2, NC_], F32, name="pg", tag="pg")
                for i in (0, 1):
                    n = 2 * np_ + i
                    for kk in range(0, T, 2):
                        nc.tensor.matmul(
                            pg[:, i, :],
                            wv[:, kk : kk + 2, t * P : (t + 1) * P],
                            xt8[:, kk : kk + 2, n * NC_ : (n + 1) * NC_],
                            start=(kk == 0), stop=(kk == T - 2), perf_mode=DR,
                        )
                ut = chk_pool.tile([P, 2, NC_], BF16, name="chk", tag="chk")
                if t == T - 1 and np_ == NP2 - 1:
                    # final pair: evacuate halves on both engines in parallel
                    # and let each rt matmul chase its own half — shortens
                    # the end-of-kernel drain chain
                    nc.scalar.activation(
                        ut[:, 0, :], pg[:, 0, :], AF.Identity,
                        bias=bv64[:, t : t + 1], scale=1.0,
                    )
                    nc.vector.tensor_scalar_add(
                        ut[:, 1, :], pg[:, 1, :], bv64[:, t : t + 1]
                    )
                else:
                    nc.scalar.activation(
                        ut, pg, AF.Identity, bias=bv64[:, t : t + 1], scale=1.0
                    )
                if vpend is not None:
                    v_tail(*vpend)
                vpend = (t, np_, ut, eff)
        v_tail(*vpend)

    nc.compile()
    return nc


def _prep_shared(inputs):
    """Host-side prep of the replicated (weight) arrays."""
    sc = 0.125  # 1/sqrt(HD)

    def rep_logit(w):
        m = np.zeros((P, P), dtype=np.float32)
        ws = w.astype(np.float32) * sc
        m[:HD, :HD] = ws[:, None]          # rows d 0..63 -> head-0 columns
        m[HD:, HD:] = ws[:, None]          # rows d 64..127 -> head-1 columns
        return m.astype(BF)

    def bias_pp(b):
        return np.ascontiguousarray(b.astype(np.float32).reshape(T, P).T)

    wrr = np.zeros((P, P), dtype=np.float32)
    wr = inputs["Wr"].astype(np.float32)
    wrr[:HD, :HD] = wr
    wrr[HD:, HD:] = wr

    br_col = np.tile(inputs["br"].astype(np.float32), 2).reshape(P, 1)

    # [k*P+p, t*P+j] -> [p][t][k][j]
    wqt = np.ascontiguousarray(
        inputs["Wq"].astype(BF).reshape(T, P, T, P).transpose(1, 2, 0, 3)
    ).reshape(P, T * T * P)

    return {
        "wq": wqt,
        "wk": np.ascontiguousarray(
            (inputs["Wk"].astype(np.float32) * W8SCALE).astype(F8)
        ),
        "wv": np.ascontiguousarray(
            (inputs["Wv"].astype(np.float32) * W8SCALE).astype(F8)
        ),
        "bqbr": bias_pp(inputs["bq"]) + br_col,
        "bk64": bias_pp(inputs["bk"]) * np.float32(W8SCALE),
        "bv64": bias_pp(inputs["bv"]) * np.float32(W8SCALE),
        "br64": np.ascontiguousarray(br_col * np.float32(INV_W8)),
        "wqlrep": rep_logit(inputs["wql"]),
        "wklrep": rep_logit(inputs["wkl"]),
        "wrr": wrr.astype(BF),
    }


def _get_nc():
    if "nc" not in _CACHE:
        _CACHE["nc"] = _build()
    return _CACHE["nc"]


def _run(inputs, trace=False):
    nc = _get_nc()
    shared = _prep_shared(inputs)
    X = inputs["X"]
    in_maps = []
    for b in range(N_CORES):
        m = dict(shared)
        xtb = np.ascontiguousarray(X[b].T)
        m["xt"] = xtb.astype(BF)
        m["xt8"] = xtb.astype(F8)
        in_maps.append(m)
    if trace:
        _install_profile_hook()
    res = run_bass_kernel_spmd(nc, in_maps, list(range(N_CORES)), trace=trace)
    out = np.empty((B, S, D), dtype=np.float32)
    for b in range(N_CORES):
        out[b] = np.asarray(res.results[b]["out"]).astype(np.float32).T
    return out, res


def _install_profile_hook():
    import antenv

    if "antenv.axon_hooks" not in sys.modules:
        mod = types.ModuleType("antenv.axon_hooks")
        mod._hook = None
        mod.set_axon_ntff_profile_hook = lambda h: setattr(mod, "_hook", h)
        mod.get_axon_ntff_profile_hook = lambda: mod._hook
        sys.modules["antenv.axon_hooks"] = mod
        antenv.axon_hooks = mod
    hooks = sys.modules["antenv.axon_hooks"]
    if hooks.get_axon_ntff_profile_hook() is None:
        from trn_agent_boot.trn_boot import _ntff_profile_via_ctypes

        hooks.set_axon_ntff_profile_hook(
            _ntff_profile_via_ctypes("/opt/axon/libaxon_pjrt.so")
        )
    import concourse.bass_utils as bass_utils

    bass_utils.upload_artifacts = lambda tmpdir: f"local:{tmpdir}"


def kernel(**inputs) -> np.ndarray:
    out, _ = _run(inputs, trace=False)
    return out


# revision 60
# speedup vs baseline: 1.0139x; 1.0139x over previous
"""Trainium2 Bass kernel for nn_AdditiveAttention (B=8, S=4096, D=1024, H=16).

Sharding: pure data-parallel over batch — 8 NeuronCores, one batch element
per core, weights replicated. No collectives.

v2 layout (everything transposed: d on partitions, s on free):
  - Q GEMM is n-outer (s-chunk outer, t-tile inner) so each xt s-chunk is
    dead right after its 8 output tiles are computed; q (bf16, +bq+br folded
    in) is written back into the xt chunk buffers with a one-chunk lag.
    Only one spare [128, 8, 512] buffer is needed for q chunk 0.
  - Per-chunk softmax pooling: logit matmul emitted one GEMM-slot late so
    PSUM evacuation always overlaps the next chunk's matmuls; exp+denominator
    fused on ScalarE (accum_out); numerator via one fused DVE
    tensor_tensor_reduce per chunk. No full-S e/p/u tiles anywhere.
  - K/V GEMMs in fp8 DoubleRow (weights host-scaled by 64; the 1/64 rides in
    the pooled-vector normalization), per-chunk gated logits / rt matmuls
    pipelined the same way.
  - Single bf16 output: out = q + (u @ Wr) (+bq+br already inside q),
    residual added during rt-PSUM evacuation on DVE. 8.4 MB written instead
    of the baseline's 33.6 MB f32 out+out2 pair.
  - wk/wv/xt8 prefetched on the scalar queue during the Q phase; startup
    loads are chunk-granular ([128,512]) and ordered so chunk 0 + wq arrive
    first on two issuing queues.
"""

import sys
import types

import numpy as np
import ml_dtypes

from contextlib import ExitStack

import concourse.bass as bass
import concourse.tile as tile
from concourse import bacc, mybir
from concourse.bass_utils import run_bass_kernel_spmd

B, S, D, H, HD = 8, 4096, 1024, 16, 64
P = 128          # partitions
T = D // P       # 8 d-tiles
NC_ = 512        # psum chunk free size
NS = S // NC_    # 8 s-chunks
N_CORES = 8
BF16 = mybir.dt.bfloat16
FP8 = mybir.dt.float8e4
F32 = mybir.dt.float32
W8SCALE = 64.0   # host scales Wk/Wv by this into e4m3 normal range
INV_W8 = 1.0 / W8SCALE
BF = ml_dtypes.bfloat16
F8 = ml_dtypes.float8_e4m3
OUT_DT = BF16  # bf16 halves output DMA traffic; host upcasts
# Pooled-sum (softmax numerator) implementation:
#   'stt_gpsimd': fused (e*(1/64))*src + accum via scalar_tensor_tensor on GpSimd
#   'stt_vector': same fused op on VectorE
#   'split':     tensor_tensor mult + reduce_sum, both on VectorE
POOL_MODE = "stt_vector"

_CACHE = {}


def _build():
    nc = bacc.Bacc(
        "TRN2", target_bir_lowering=False, debug=False, num_devices=N_CORES
    )
    xt_ext = nc.declare_dram_parameter("xt", [D, S], BF16, isOutput=False)
    xt8_ext = nc.declare_dram_parameter("xt8", [D, S], FP8, isOutput=False)
    # wq arrives pre-packed by output-tile column blocks: row p holds
    # [t][k][j] so block t (the stationaries for output tile t, all k) is a
    # contiguous 256KB slab — loadable incrementally in t order
    wq_ext = nc.declare_dram_parameter("wq", [P, T * T * P], BF16, isOutput=False)
    wk_ext = nc.declare_dram_parameter("wk", [D, D], FP8, isOutput=False)
    wv_ext = nc.declare_dram_parameter("wv", [D, D], FP8, isOutput=False)
    bqbr_ext = nc.declare_dram_parameter("bqbr", [P, T], F32, isOutput=False)
    bk_ext = nc.declare_dram_parameter("bk64", [P, T], F32, isOutput=False)
    bv_ext = nc.declare_dram_parameter("bv64", [P, T], F32, isOutput=False)
    br64_ext = nc.declare_dram_parameter("br64", [P, 1], F32, isOutput=False)
    wql_ext = nc.declare_dram_parameter("wqlrep", [P, P], BF16, isOutput=False)
    wkl_ext = nc.declare_dram_parameter("wklrep", [P, P], BF16, isOutput=False)
    wrr_ext = nc.declare_dram_parameter("wrr", [P, P], BF16, isOutput=False)
    out_ext = nc.declare_dram_parameter("out", [D, S], OUT_DT, isOutput=True)

    AX = mybir.AxisListType.X
    ALU = mybir.AluOpType
    AF = mybir.ActivationFunctionType
    DR = mybir.MatmulPerfMode.DoubleRow

    with tile.TileContext(nc) as tc, ExitStack() as ctx:
        singles = ctx.enter_context(tc.tile_pool(name="singles", bufs=1))
        psg = ctx.enter_context(tc.tile_pool(name="psg", bufs=2, space="PSUM"))
        psl = ctx.enter_context(tc.tile_pool(name="psl", bufs=2, space="PSUM"))
        chk_pool = ctx.enter_context(tc.tile_pool(name="chk", bufs=4))
        e_pool = ctx.enter_context(tc.tile_pool(name="epool", bufs=2))
        m_pool = ctx.enter_context(tc.tile_pool(name="mpool", bufs=2))
        # stg depth decouples the V-phase residual adds (and through PSUM
        # WAR, the rt matmuls) from out-DMA completion pacing
        stg_pool = ctx.enter_context(tc.tile_pool(name="stg", bufs=8))
        eff_pool = ctx.enter_context(tc.tile_pool(name="eff", bufs=2))
        small_pool = ctx.enter_context(tc.tile_pool(name="small", bufs=2))

        # ---- resident tiles ----
        xtc = [
            singles.tile([P, T, NC_], BF16, name=f"xtc{n}", tag=f"xtc{n}")
            for n in range(NS)
        ]
        qsp = singles.tile([P, T, NC_], BF16, name="qsp", tag="qsp")
        xt8 = singles.tile([P, T, S], FP8, name="xt8", tag="xt8")
        wq = singles.tile([P, T, T, P], BF16, name="wq", tag="wq")
        wk = singles.tile([P, T, D], FP8, name="wk", tag="wk")
        wv = singles.tile([P, T, D], FP8, name="wv", tag="wv")
        wqlrep = singles.tile([P, P], BF16, name="wqlrep", tag="wqlrep")
        wklrep = singles.tile([P, P], BF16, name="wklrep", tag="wklrep")
        wrr = singles.tile([P, P], BF16, name="wrr", tag="wrr")
        bqbr = singles.tile([P, T], F32, name="bqbr", tag="bqbr")
        bk64 = singles.tile([P, T], F32, name="bk64", tag="bk64")
        bv64 = singles.tile([P, T], F32, name="bv64", tag="bv64")
        br64 = singles.tile([P, 1], F32, name="br64", tag="br64")
        gq_all = singles.tile([P, T], F32, name="gq", tag="gq")
        gk_all = singles.tile([P, T], F32, name="gk", tag="gk")
        pe_q = singles.tile([P, T * NS], F32, name="peq", tag="peq")
        pq_q = singles.tile([P, T * NS], F32, name="pqq", tag="pqq")
        pe_k = singles.tile([P, T * (NS // 2)], F32, name="pek", tag="pek")
        pq_k = singles.tile([P, T * (NS // 2)], F32, name="pqk", tag="pqk")

        pace_sem = nc.alloc_semaphore("pace_sem")

        # ---- DMA issue (ordering matters for startup) ----
        # Chunk 0 goes first, split across both queues, so the first GEMM's
        # data is in front of everything; then wq (alternating), then the
        # remaining chunks with each queue carrying half of every chunk so
        # both queues advance in lockstep with TensorE's chunk consumption.
        def xtc_dma(eng, n, k):
            eng.dma_start(
                xtc[n][:, k, :],
                xt_ext.ap()[k * P : (k + 1) * P, n * NC_ : (n + 1) * NC_],
            )

        def xtc_dma_half(eng, n, k, h):
            hw = NC_ // 2
            eng.dma_start(
                xtc[n][:, k, h * hw : (h + 1) * hw],
                xt_ext.ap()[
                    k * P : (k + 1) * P,
                    n * NC_ + h * hw : n * NC_ + (h + 1) * hw,
                ],
            )

        # wq t-blocks stream in t order, interleaved with the first two xt
        # chunks so supply tracks the Q loop's (chunk, t)-sweep demand.
        # Early transfers are 64KB quarters: one DMA occupies one ring
        # (~22GB/s), so parallelism across rings comes from DMA count.
        QB = T * P // 4  # quarter-block columns

        def wq_dma(t, fine=False):
            base = t * T * P
            if fine:
                for qtr in range(4):
                    eng = nc.sync if qtr < 2 else nc.gpsimd
                    eng.dma_start(
                        wq[:, t, qtr * 2 : qtr * 2 + 2, :],
                        wq_ext.ap()[:, base + qtr * QB : base + (qtr + 1) * QB],
                    )
            else:
                nc.sync.dma_start(
                    wq[:, t, : T // 2, :], wq_ext.ap()[:, base : base + 2 * QB]
                )
                nc.gpsimd.dma_start(
                    wq[:, t, T // 2 :, :],
                    wq_ext.ap()[:, base + 2 * QB : base + T * P],
                )

        wq_dma(0, fine=True)
        for k in range(T):
            xtc_dma(nc.sync if k < T // 2 else nc.gpsimd, 0, k)
        for t in (1, 2, 3):
            wq_dma(t)
        for k in range(3):
            xtc_dma(nc.sync, 1, k)
        for k in range(3, 6):
            xtc_dma(nc.gpsimd, 1, k)
        for t in range(4, T):
            wq_dma(t)
        # scalar: small weights/biases, then K/V weights + fp8 X (all needed
        # only from the K phase on).
        nc.scalar.dma_start(wqlrep[:], wql_ext.ap())
        nc.scalar.dma_start(bqbr[:], bqbr_ext.ap())
        nc.scalar.dma_start(br64[:], br64_ext.ap())
        nc.scalar.dma_start(wklrep[:], wkl_ext.ap())
        nc.scalar.dma_start(wrr[:], wrr_ext.ap())
        nc.scalar.dma_start(bk64[:], bk_ext.ap())
        nc.scalar.dma_start(bv64[:], bv_ext.ap())
        # chunk 1's last slices on the scalar queue (issued immediately)
        for k in (6, 7):
            xtc_dma(nc.scalar, 1, k)
        # Chunks 2..7 are PACED: the DMA rings serve queued descriptors
        # round-robin, so issuing everything up front makes every chunk
        # finish "evenly late". Gate chunk m's issue on chunk m-2's compute
        # (scalar bumps pace_sem at each Q chunk boundary) so at most ~2
        # chunks of loads compete for the rings at once.
        for n in range(2, NS):
            nc.sync.wait_ge(pace_sem, n - 2)
            for k in range(3):
                xtc_dma(nc.sync, n, k)
            nc.gpsimd.wait_ge(pace_sem, n - 2)
            for k in range(3, 6):
                xtc_dma(nc.gpsimd, n, k)

        def kv_prefetch(n):
            """K/V-phase loads, paced: issued on the scalar queue at chunk-n
            boundaries of the Q loop so they don't contend with the Q-phase
            chunk streaming that feeds TensorE."""
            if n == 1:
                for k in range(T):
                    nc.scalar.dma_start(
                        wk[:, k, :], wk_ext.ap()[k * P : (k + 1) * P, :]
                    )
            elif n == 2:
                for k in range(T):
                    nc.scalar.dma_start(
                        wv[:, k, :], wv_ext.ap()[k * P : (k + 1) * P, :]
                    )
            elif 3 <= n <= 6:
                for k in (2 * (n - 3), 2 * (n - 3) + 1):
                    rsl = slice(k * P, (k + 1) * P)
                    nc.scalar.dma_start(
                        xt8[:, k, : S // 2], xt8_ext.ap()[rsl, : S // 2]
                    )
                    nc.scalar.dma_start(
                        xt8[:, k, S // 2 :], xt8_ext.ap()[rsl, S // 2 :]
                    )

        def qreg_of(t, n):
            """q chunk n of tile t lives in xt chunk n-1's space (spare for
            n=0)."""
            src = qsp if n == 0 else xtc[n - 1]
            return src[:, t, :]

        def pool_sum(e, src, accum_col):
            """accum_col = sum_s e[:,s]*src[:,s] / 64 (the 1/64 un-scales the
            fp8 K/V weight scaling; for Q it cancels in num/denom)."""
            m = m_pool.tile(list(e.shape), BF16, name="m", tag="m")
            if POOL_MODE == "stt_vector":
                nc.vector.scalar_tensor_tensor(
                    m, e, INV_W8, src, op0=ALU.mult, op1=ALU.mult,
                    accum_out=accum_col,
                )
            else:
                nc.vector.tensor_tensor(m, e, src, ALU.mult)
                nc.vector.reduce_sum(accum_col, m, axis=AX)

        # ---- Q phase: n-outer so xt chunks free up for q storage ----
        # GEMM and logit PSUM tiles are 2-bank pairs (halves used per slot)
        # so psg+psl fit the 8 PSUM banks alongside the K/V phases' paired
        # layout.
        # PE p-state warmup: the array runs 2-2.7x slow for the first ~20us
        # after going busy. Burn dummy matmuls on junk SBUF during the
        # 6-12us window where TensorE would otherwise idle waiting for the
        # first loads, so real work starts at full clock.
        warm_src = singles.tile([P, NC_], BF16, name="warm", tag="warm")
        nc.vector.memset(warm_src[:], 1.0)
        warm_ps = psl.tile([P, 2, NC_], F32, name="pl", tag="pl")
        for w in range(12):
            nc.tensor.matmul(
                warm_ps[:, w % 2, :], warm_src[:, :P], warm_src[:],
                start=True, stop=True,
            )

        qpl_state = {}

        def q_tail(t, n):
            """Delayed-by-one-slot logit matmul + exp + pooled partials."""
            qreg = qreg_of(t, n)
            slot = n * T + t
            if slot % 2 == 0:
                qpl_state["pl"] = psl.tile([P, 2, NC_], F32, name="pl", tag="pl")
            pl = qpl_state["pl"][:, slot % 2, :]
            nc.tensor.matmul(pl, wqlrep[:], qreg, start=True, stop=True)
            col = slice(t * NS + n, t * NS + n + 1)
            e = e_pool.tile([P, NC_], BF16, name="e", tag="e")
            nc.scalar.activation(
                e, pl, AF.Exp, bias=0.0, scale=1.0, accum_out=pe_q[:, col]
            )
            pool_sum(e, qreg, pq_q[:, col])

        pend = None
        qpg = None
        for n in range(NS):
            for t in range(T):
                slot = n * T + t
                if slot % 2 == 0:
                    qpg = psg.tile([P, 2, NC_], F32, name="pg", tag="pg")
                pch = qpg[:, slot % 2, :]
                for k in range(T):
                    nc.tensor.matmul(
                        pch, wq[:, t, k, :], xtc[n][:, k, :],
                        start=(k == 0), stop=(k == T - 1),
                    )
                qreg = qreg_of(t, n)
                if slot % 2 == 0:
                    nc.scalar.activation(
                        qreg, pch, AF.Identity, bias=bqbr[:, t : t + 1], scale=1.0
                    )
                else:
                    nc.vector.tensor_scalar_add(qreg, pch, bqbr[:, t : t + 1])
                if pend is not None:
                    q_tail(*pend)
                pend = (t, n)
            # scalar reaches these after chunk n's evac/exp work: paced issue
            nc.scalar.sem_inc(pace_sem, 1)
            if n + 2 < NS:
                for k in (6, 7):
                    xtc_dma(nc.scalar, n + 2, k)
            kv_prefetch(n)
        q_tail(*pend)

        # Q pool finalizers: gq_all = gq_true/64 (br contribution removed).
        for t in range(T):
            tsl = slice(t * NS, (t + 1) * NS)
            stot = small_pool.tile([P, 1], F32, name="stot", tag="stot")
            nc.vector.reduce_sum(stot, pe_q[:, tsl], axis=AX)
            rec = small_pool.tile([P, 1], F32, name="rec", tag="rec")
            nc.vector.reciprocal(rec, stot)
            if POOL_MODE == "split":
                nc.vector.tensor_scalar_mul(rec, rec, INV_W8)
            graw = small_pool.tile([P, 1], F32, name="graw", tag="graw")
            nc.vector.reduce_sum(graw, pq_q[:, tsl], axis=AX)
            tmp = small_pool.tile([P, 1], F32, name="gtmp", tag="gtmp")
            nc.vector.tensor_tensor(tmp, graw, rec, ALU.mult)
            nc.vector.tensor_tensor(gq_all[:, t : t + 1], tmp, br64[:], ALU.subtract)

        # ---- K phase: t-outer, chunk-PAIR pipelined gated logits ----
        # One 2-bank GEMM psum pair per two s-chunks; a single evacuation,
        # exp+denominator, and fused pooled-sum op each cover the whole pair,
        # halving DVE/ACT instruction counts so both stay under TensorE's
        # 1080ns/slot fp8 pace.
        NP2 = NS // 2

        def k_tail(t, np_, pt, eff):
            pl = psl.tile([P, 2, NC_], F32, name="pl", tag="pl")
            nc.tensor.matmul(pl[:, 0, :], eff[:], pt[:, 0, :], start=True, stop=True)
            nc.tensor.matmul(pl[:, 1, :], eff[:], pt[:, 1, :], start=True, stop=True)
            col = slice(t * NP2 + np_, t * NP2 + np_ + 1)
            e = e_pool.tile([P, 2, NC_], BF16, name="e", tag="e")
            nc.scalar.activation(
                e, pl, AF.Exp, bias=0.0, scale=1.0, accum_out=pe_k[:, col]
            )
            pool_sum(e, pt, pq_k[:, col])

        def k_final(t):
            tsl = slice(t * NP2, (t + 1) * NP2)
            stot = small_pool.tile([P, 1], F32, name="stot", tag="stot")
            nc.vector.reduce_sum(stot, pe_k[:, tsl], axis=AX)
            rec = small_pool.tile([P, 1], F32, name="rec", tag="rec")
            nc.vector.reciprocal(rec, stot)
            if POOL_MODE == "split":
                nc.vector.tensor_scalar_mul(rec, rec, INV_W8)
            graw = small_pool.tile([P, 1], F32, name="graw", tag="graw")
            nc.vector.reduce_sum(graw, pq_k[:, tsl], axis=AX)
            tmp = small_pool.tile([P, 1], F32, name="gtmp", tag="gtmp")
            nc.vector.tensor_tensor(tmp, graw, rec, ALU.mult)
            nc.vector.tensor_tensor(
                gk_all[:, t : t + 1], tmp, gq_all[:, t : t + 1], ALU.mult
            )

        kpend = None
        for t in range(T):
            eff = eff_pool.tile([P, P], BF16, name="effkl", tag="eff")
            nc.vector.tensor_scalar_mul(eff[:], wklrep[:], gq_all[:, t : t + 1])
            for np_ in range(NP2):
                pg = psg.tile([P, 2, NC_], F32, name="pg", tag="pg")
                for i in (0, 1):
                    n = 2 * np_ + i
                    for kk in range(0, T, 2):
                        nc.tensor.matmul(
                            pg[:, i, :],
                            wk[:, kk : kk + 2, t * P : (t + 1) * P],
                            xt8[:, kk : kk + 2, n * NC_ : (n + 1) * NC_],
                            start=(kk == 0), stop=(kk == T - 2), perf_mode=DR,
                        )
                pt = chk_pool.tile([P, 2, NC_], BF16, name="chk", tag="chk")
                if np_ % 2 == 0:
                    nc.scalar.activation(
                        pt, pg, AF.Identity, bias=bk64[:, t : t + 1], scale=1.0
                    )
                else:
                    nc.vector.tensor_scalar_add(pt, pg, bk64[:, t : t + 1])
                if kpend is not None:
                    k_tail(*kpend)
                    if kpend[1] == NP2 - 1:
                        k_final(kpend[0])
                kpend = (t, np_, pt, eff)
        k_tail(*kpend)
        k_final(T - 1)

        # ---- V phase: chunk-pair rt matmuls + residual add + store ----
        def v_tail(t, np_, ut, eff):
            pl = psl.tile([P, 2, NC_], F32, name="pl", tag="pl")
            for i in (0, 1):
                n = 2 * np_ + i
                nc.tensor.matmul(
                    pl[:, i, :], eff[:], ut[:, i, :], start=True, stop=True
                )
                stg = stg_pool.tile([P, NC_], OUT_DT, name="stg", tag="stg")
                nc.vector.tensor_tensor(stg, pl[:, i, :], qreg_of(t, n), ALU.add)
                osl = slice(t * P, (t + 1) * P)
                csl = slice(n * NC_, (n + 1) * NC_)
                if t < T - 1:
                    dma_eng = nc.sync if n % 2 == 0 else nc.gpsimd
                    dma_eng.dma_start(out_ext.ap()[osl, csl], stg)
                else:
                    # final tile: split each store across the two DMA queues
                    # that have no compute left so the last transfers drain
                    # on multiple rings
                    h = NC_ // 2
                    nc.sync.dma_start(
                        out_ext.ap()[osl, n * NC_ : n * NC_ + h], stg[:, :h]
                    )
                    nc.gpsimd.dma_start(
                        out_ext.ap()[osl, n * NC_ + h : (n + 1) * NC_], stg[:, h:]
                    )

        vpend = None
        for t in range(T):
            eff = eff_pool.tile([P, P], BF16, name="effrt", tag="eff")
            nc.vector.tensor_scalar_mul(eff[:], wrr[:], gk_all[:, t : t + 1])
            for np_ in range(NP2):
                pg = psg.tile([P, 2, NC_], F32, name="pg", tag="pg")
                for i in (0, 1):
                    n = 2 * np_ + i
                    for kk in range(0, T, 2):
                        nc.tensor.matmul(
                            pg[:, i, :],
                            wv[:, kk : kk + 2, t * P : (t + 1) * P],
                            xt8[:, kk : kk + 2, n * NC_ : (n + 1) * NC_],
                            start=(kk == 0), stop=(kk == T - 2), perf_mode=DR,
                        )
                ut = chk_pool.tile([P, 2, NC_], BF16, name="chk", tag="chk")
                nc.scalar.activation(
                    ut, pg, AF.Identity, bias=bv64[:, t : t + 1], scale=1.0
                )
                if vpend is not None:
                    v_tail(*vpend)
                vpend = (t, np_, ut, eff)
        v_tail(*vpend)

    nc.compile()
    return nc


def _prep_shared(inputs):
    """Host-side prep of the replicated (weight) arrays."""
    sc = 0.125  # 1/sqrt(HD)

    def rep_logit(w):
        m = np.zeros((P, P), dtype=np.float32)
        ws = w.astype(np.float32) * sc
        m[:HD, :HD] = ws[:, None]          # rows d 0..63 -> head-0 columns
        m[HD:, HD:] = ws[:, None]          # rows d 64..127 -> head-1 columns
        return m.astype(BF)

    def bias_pp(b):
        return np.ascontiguousarray(b.astype(np.float32).reshape(T, P).T)

    wrr = np.zeros((P, P), dtype=np.float32)
    wr = inputs["Wr"].astype(np.float32)
    wrr[:HD, :HD] = wr
    wrr[HD:, HD:] = wr

    br_col = np.tile(inputs["br"].astype(np.float32), 2).reshape(P, 1)

    # [k*P+p, t*P+j] -> [p][t][k][j]
    wqt = np.ascontiguousarray(
        inputs["Wq"].astype(BF).reshape(T, P, T, P).transpose(1, 2, 0, 3)
    ).reshape(P, T * T * P)

    return {
        "wq": wqt,
        "wk": np.ascontiguousarray(
            (inputs["Wk"].astype(np.float32) * W8SCALE).astype(F8)
        ),
        "wv": np.ascontiguousarray(
            (inputs["Wv"].astype(np.float32) * W8SCALE).astype(F8)
        ),
        "bqbr": bias_pp(inputs["bq"]) + br_col,
        "bk64": bias_pp(inputs["bk"]) * np.float32(W8SCALE),
        "bv64": bias_pp(inputs["bv"]) * np.float32(W8SCALE),
        "br64": np.ascontiguousarray(br_col * np.float32(INV_W8)),
        "wqlrep": rep_logit(inputs["wql"]),
        "wklrep": rep_logit(inputs["wkl"]),
        "wrr": wrr.astype(BF),
    }


def _get_nc():
    if "nc" not in _CACHE:
        _CACHE["nc"] = _build()
    return _CACHE["nc"]


def _run(inputs, trace=False):
    nc = _get_nc()
    shared = _prep_shared(inputs)
    X = inputs["X"]
    in_maps = []
    for b in range(N_CORES):
        m = dict(shared)
        xtb = np.ascontiguousarray(X[b].T)
        m["xt"] = xtb.astype(BF)
        m["xt8"] = xtb.astype(F8)
        in_maps.append(m)
    if trace:
        _install_profile_hook()
    res = run_bass_kernel_spmd(nc, in_maps, list(range(N_CORES)), trace=trace)
    out = np.empty((B, S, D), dtype=np.float32)
    for b in range(N_CORES):
        out[b] = np.asarray(res.results[b]["out"]).astype(np.float32).T
    return out, res


def _install_profile_hook():
    import antenv

    if "antenv.axon_hooks" not in sys.modules:
        mod = types.ModuleType("antenv.axon_hooks")
        mod._hook = None
        mod.set_axon_ntff_profile_hook = lambda h: setattr(mod, "_hook", h)
        mod.get_axon_ntff_profile_hook = lambda: mod._hook
        sys.modules["antenv.axon_hooks"] = mod
        antenv.axon_hooks = mod
    hooks = sys.modules["antenv.axon_hooks"]
    if hooks.get_axon_ntff_profile_hook() is None:
        from trn_agent_boot.trn_boot import _ntff_profile_via_ctypes

        hooks.set_axon_ntff_profile_hook(
            _ntff_profile_via_ctypes("/opt/axon/libaxon_pjrt.so")
        )
    import concourse.bass_utils as bass_utils

    bass_utils.upload_artifacts = lambda tmpdir: f"local:{tmpdir}"


def kernel(**inputs) -> np.ndarray:
    out, _ = _run(inputs, trace=False)
    return out


# revision 62
# speedup vs baseline: 1.0179x; 1.0039x over previous
"""Trainium2 Bass kernel for nn_AdditiveAttention (B=8, S=4096, D=1024, H=16).

Sharding: pure data-parallel over batch — 8 NeuronCores, one batch element
per core, weights replicated. No collectives.

v2 layout (everything transposed: d on partitions, s on free):
  - Q GEMM is n-outer (s-chunk outer, t-tile inner) so each xt s-chunk is
    dead right after its 8 output tiles are computed; q (bf16, +bq+br folded
    in) is written back into the xt chunk buffers with a one-chunk lag.
    Only one spare [128, 8, 512] buffer is needed for q chunk 0.
  - Per-chunk softmax pooling: logit matmul emitted one GEMM-slot late so
    PSUM evacuation always overlaps the next chunk's matmuls; exp+denominator
    fused on ScalarE (accum_out); numerator via one fused DVE
    tensor_tensor_reduce per chunk. No full-S e/p/u tiles anywhere.
  - K/V GEMMs in fp8 DoubleRow (weights host-scaled by 64; the 1/64 rides in
    the pooled-vector normalization), per-chunk gated logits / rt matmuls
    pipelined the same way.
  - Single bf16 output: out = q + (u @ Wr) (+bq+br already inside q),
    residual added during rt-PSUM evacuation on DVE. 8.4 MB written instead
    of the baseline's 33.6 MB f32 out+out2 pair.
  - wk/wv/xt8 prefetched on the scalar queue during the Q phase; startup
    loads are chunk-granular ([128,512]) and ordered so chunk 0 + wq arrive
    first on two issuing queues.
"""

import sys
import types

import numpy as np
import ml_dtypes

from contextlib import ExitStack

import concourse.bass as bass
import concourse.tile as tile
from concourse import bacc, mybir
from concourse.bass_utils import run_bass_kernel_spmd

B, S, D, H, HD = 8, 4096, 1024, 16, 64
P = 128          # partitions
T = D // P       # 8 d-tiles
NC_ = 512        # psum chunk free size
NS = S // NC_    # 8 s-chunks
N_CORES = 8
BF16 = mybir.dt.bfloat16
FP8 = mybir.dt.float8e4
F32 = mybir.dt.float32
W8SCALE = 64.0   # host scales Wk/Wv by this into e4m3 normal range
INV_W8 = 1.0 / W8SCALE
BF = ml_dtypes.bfloat16
F8 = ml_dtypes.float8_e4m3
OUT_DT = BF16  # bf16 halves output DMA traffic; host upcasts
# Pooled-sum (softmax numerator) implementation:
#   'stt_gpsimd': fused (e*(1/64))*src + accum via scalar_tensor_tensor on GpSimd
#   'stt_vector': same fused op on VectorE
#   'split':     tensor_tensor mult + reduce_sum, both on VectorE
POOL_MODE = "stt_vector"

_CACHE = {}


def _build():
    nc = bacc.Bacc(
        "TRN2", target_bir_lowering=False, debug=False, num_devices=N_CORES
    )
    xt_ext = nc.declare_dram_parameter("xt", [D, S], BF16, isOutput=False)
    xt8_ext = nc.declare_dram_parameter("xt8", [D, S], FP8, isOutput=False)
    # wq arrives pre-packed by output-tile column blocks: row p holds
    # [t][k][j] so block t (the stationaries for output tile t, all k) is a
    # contiguous 256KB slab — loadable incrementally in t order
    wq_ext = nc.declare_dram_parameter("wq", [P, T * T * P], BF16, isOutput=False)
    wk_ext = nc.declare_dram_parameter("wk", [D, D], FP8, isOutput=False)
    wv_ext = nc.declare_dram_parameter("wv", [D, D], FP8, isOutput=False)
    bqbr_ext = nc.declare_dram_parameter("bqbr", [P, T], F32, isOutput=False)
    bk_ext = nc.declare_dram_parameter("bk64", [P, T], F32, isOutput=False)
    bv_ext = nc.declare_dram_parameter("bv64", [P, T], F32, isOutput=False)
    br64_ext = nc.declare_dram_parameter("br64", [P, 1], F32, isOutput=False)
    wql_ext = nc.declare_dram_parameter("wqlrep", [P, P], BF16, isOutput=False)
    wkl_ext = nc.declare_dram_parameter("wklrep", [P, P], BF16, isOutput=False)
    wrr_ext = nc.declare_dram_parameter("wrr", [P, P], BF16, isOutput=False)
    out_ext = nc.declare_dram_parameter("out", [D, S], OUT_DT, isOutput=True)

    AX = mybir.AxisListType.X
    ALU = mybir.AluOpType
    AF = mybir.ActivationFunctionType
    DR = mybir.MatmulPerfMode.DoubleRow

    with tile.TileContext(nc) as tc, ExitStack() as ctx:
        singles = ctx.enter_context(tc.tile_pool(name="singles", bufs=1))
        psg = ctx.enter_context(tc.tile_pool(name="psg", bufs=2, space="PSUM"))
        psl = ctx.enter_context(tc.tile_pool(name="psl", bufs=2, space="PSUM"))
        chk_pool = ctx.enter_context(tc.tile_pool(name="chk", bufs=4))
        e_pool = ctx.enter_context(tc.tile_pool(name="epool", bufs=2))
        m_pool = ctx.enter_context(tc.tile_pool(name="mpool", bufs=2))
        # stg depth decouples the V-phase residual adds (and through PSUM
        # WAR, the rt matmuls) from out-DMA completion pacing
        stg_pool = ctx.enter_context(tc.tile_pool(name="stg", bufs=8))
        eff_pool = ctx.enter_context(tc.tile_pool(name="eff", bufs=2))
        small_pool = ctx.enter_context(tc.tile_pool(name="small", bufs=2))

        # ---- resident tiles ----
        xtc = [
            singles.tile([P, T, NC_], BF16, name=f"xtc{n}", tag=f"xtc{n}")
            for n in range(NS)
        ]
        qsp = singles.tile([P, T, NC_], BF16, name="qsp", tag="qsp")
        xt8 = singles.tile([P, T, S], FP8, name="xt8", tag="xt8")
        wq = singles.tile([P, T, T, P], BF16, name="wq", tag="wq")
        wk = singles.tile([P, T, D], FP8, name="wk", tag="wk")
        wv = singles.tile([P, T, D], FP8, name="wv", tag="wv")
        wqlrep = singles.tile([P, P], BF16, name="wqlrep", tag="wqlrep")
        wklrep = singles.tile([P, P], BF16, name="wklrep", tag="wklrep")
        wrr = singles.tile([P, P], BF16, name="wrr", tag="wrr")
        bqbr = singles.tile([P, T], F32, name="bqbr", tag="bqbr")
        bk64 = singles.tile([P, T], F32, name="bk64", tag="bk64")
        bv64 = singles.tile([P, T], F32, name="bv64", tag="bv64")
        br64 = singles.tile([P, 1], F32, name="br64", tag="br64")
        gq_all = singles.tile([P, T], F32, name="gq", tag="gq")
        gk_all = singles.tile([P, T], F32, name="gk", tag="gk")
        pe_q = singles.tile([P, T * NS], F32, name="peq", tag="peq")
        pq_q = singles.tile([P, T * NS], F32, name="pqq", tag="pqq")
        pe_k = singles.tile([P, T * (NS // 2)], F32, name="pek", tag="pek")
        pq_k = singles.tile([P, T * (NS // 2)], F32, name="pqk", tag="pqk")

        pace_sem = nc.alloc_semaphore("pace_sem")

        # ---- DMA issue (ordering matters for startup) ----
        # Chunk 0 goes first, split across both queues, so the first GEMM's
        # data is in front of everything; then wq (alternating), then the
        # remaining chunks with each queue carrying half of every chunk so
        # both queues advance in lockstep with TensorE's chunk consumption.
        def xtc_dma(eng, n, k):
            eng.dma_start(
                xtc[n][:, k, :],
                xt_ext.ap()[k * P : (k + 1) * P, n * NC_ : (n + 1) * NC_],
            )

        def xtc_dma_half(eng, n, k, h):
            hw = NC_ // 2
            eng.dma_start(
                xtc[n][:, k, h * hw : (h + 1) * hw],
                xt_ext.ap()[
                    k * P : (k + 1) * P,
                    n * NC_ + h * hw : n * NC_ + (h + 1) * hw,
                ],
            )

        # wq t-blocks stream in t order, interleaved with the first two xt
        # chunks so supply tracks the Q loop's (chunk, t)-sweep demand.
        # Early transfers are 64KB quarters: one DMA occupies one ring
        # (~22GB/s), so parallelism across rings comes from DMA count.
        QB = T * P // 4  # quarter-block columns

        def wq_dma(t, fine=False):
            base = t * T * P
            if fine:
                for qtr in range(4):
                    eng = nc.sync if qtr < 2 else nc.gpsimd
                    eng.dma_start(
                        wq[:, t, qtr * 2 : qtr * 2 + 2, :],
                        wq_ext.ap()[:, base + qtr * QB : base + (qtr + 1) * QB],
                    )
            else:
                nc.sync.dma_start(
                    wq[:, t, : T // 2, :], wq_ext.ap()[:, base : base + 2 * QB]
                )
                nc.gpsimd.dma_start(
                    wq[:, t, T // 2 :, :],
                    wq_ext.ap()[:, base + 2 * QB : base + T * P],
                )

        wq_dma(0, fine=True)
        for k in range(T):
            xtc_dma(nc.sync if k < T // 2 else nc.gpsimd, 0, k)
        for t in (1, 2, 3):
            wq_dma(t)
        for k in (0, 1):
            xtc_dma(nc.sync, 1, k)
        for k in (2, 3):
            xtc_dma(nc.gpsimd, 1, k)
        for t in range(4, T):
            wq_dma(t)
        # scalar: small weights/biases, then K/V weights + fp8 X (all needed
        # only from the K phase on).
        nc.scalar.dma_start(wqlrep[:], wql_ext.ap())
        nc.scalar.dma_start(bqbr[:], bqbr_ext.ap())
        nc.scalar.dma_start(br64[:], br64_ext.ap())
        nc.scalar.dma_start(wklrep[:], wkl_ext.ap())
        nc.scalar.dma_start(wrr[:], wrr_ext.ap())
        nc.scalar.dma_start(bk64[:], bk_ext.ap())
        nc.scalar.dma_start(bv64[:], bv_ext.ap())
        # chunk 1's upper half on the scalar queue, which is otherwise idle
        # during the startup window
        for k in (4, 5, 6, 7):
            xtc_dma(nc.scalar, 1, k)
        # Chunks 2..7 are PACED: the DMA rings serve queued descriptors
        # round-robin, so issuing everything up front makes every chunk
        # finish "evenly late". Gate chunk m's issue on chunk m-2's compute
        # (scalar bumps pace_sem at each Q chunk boundary) so at most ~2
        # chunks of loads compete for the rings at once.
        for n in range(2, NS):
            nc.sync.wait_ge(pace_sem, n - 2)
            for k in range(3):
                xtc_dma(nc.sync, n, k)
            nc.gpsimd.wait_ge(pace_sem, n - 2)
            for k in range(3, 6):
                xtc_dma(nc.gpsimd, n, k)

        def kv_prefetch(n):
            """K/V-phase loads, paced: issued on the scalar queue at chunk-n
            boundaries of the Q loop so they don't contend with the Q-phase
            chunk streaming that feeds TensorE."""
            if n == 1:
                for k in range(T):
                    nc.scalar.dma_start(
                        wk[:, k, :], wk_ext.ap()[k * P : (k + 1) * P, :]
                    )
            elif n == 2:
                for k in range(T):
                    nc.scalar.dma_start(
                        wv[:, k, :], wv_ext.ap()[k * P : (k + 1) * P, :]
                    )
            elif 3 <= n <= 6:
                for k in (2 * (n - 3), 2 * (n - 3) + 1):
                    rsl = slice(k * P, (k + 1) * P)
                    nc.scalar.dma_start(
                        xt8[:, k, : S // 2], xt8_ext.ap()[rsl, : S // 2]
                    )
                    nc.scalar.dma_start(
                        xt8[:, k, S // 2 :], xt8_ext.ap()[rsl, S // 2 :]
                    )

        def qreg_of(t, n):
            """q chunk n of tile t lives in xt chunk n-1's space (spare for
            n=0)."""
            src = qsp if n == 0 else xtc[n - 1]
            return src[:, t, :]

        def pool_sum(e, src, accum_col):
            """accum_col = sum_s e[:,s]*src[:,s] / 64 (the 1/64 un-scales the
            fp8 K/V weight scaling; for Q it cancels in num/denom)."""
            m = m_pool.tile(list(e.shape), BF16, name="m", tag="m")
            if POOL_MODE == "stt_vector":
                nc.vector.scalar_tensor_tensor(
                    m, e, INV_W8, src, op0=ALU.mult, op1=ALU.mult,
                    accum_out=accum_col,
                )
            else:
                nc.vector.tensor_tensor(m, e, src, ALU.mult)
                nc.vector.reduce_sum(accum_col, m, axis=AX)

        # ---- Q phase: n-outer so xt chunks free up for q storage ----
        # GEMM and logit PSUM tiles are 2-bank pairs (halves used per slot)
        # so psg+psl fit the 8 PSUM banks alongside the K/V phases' paired
        # layout.
        # PE p-state warmup: the array runs 2-2.7x slow for the first ~20us
        # after going busy. Burn dummy matmuls on junk SBUF during the
        # 6-12us window where TensorE would otherwise idle waiting for the
        # first loads, so real work starts at full clock.
        warm_src = singles.tile([P, NC_], BF16, name="warm", tag="warm")
        nc.vector.memset(warm_src[:], 1.0)
        warm_ps = psl.tile([P, 2, NC_], F32, name="pl", tag="pl")
        for w in range(12):
            nc.tensor.matmul(
                warm_ps[:, w % 2, :], warm_src[:, :P], warm_src[:],
                start=True, stop=True,
            )

        qpl_state = {}

        def q_tail(t, n):
            """Delayed-by-one-slot logit matmul + exp + pooled partials."""
            qreg = qreg_of(t, n)
            slot = n * T + t
            if slot % 2 == 0:
                qpl_state["pl"] = psl.tile([P, 2, NC_], F32, name="pl", tag="pl")
            pl = qpl_state["pl"][:, slot % 2, :]
            nc.tensor.matmul(pl, wqlrep[:], qreg, start=True, stop=True)
            col = slice(t * NS + n, t * NS + n + 1)
            e = e_pool.tile([P, NC_], BF16, name="e", tag="e")
            nc.scalar.activation(
                e, pl, AF.Exp, bias=0.0, scale=1.0, accum_out=pe_q[:, col]
            )
            pool_sum(e, qreg, pq_q[:, col])

        pend = None
        qpg = None
        for n in range(NS):
            for t in range(T):
                slot = n * T + t
                if slot % 2 == 0:
                    qpg = psg.tile([P, 2, NC_], F32, name="pg", tag="pg")
                pch = qpg[:, slot % 2, :]
                for k in range(T):
                    nc.tensor.matmul(
                        pch, wq[:, t, k, :], xtc[n][:, k, :],
                        start=(k == 0), stop=(k == T - 1),
                    )
                qreg = qreg_of(t, n)
                if slot % 2 == 0:
                    nc.scalar.activation(
                        qreg, pch, AF.Identity, bias=bqbr[:, t : t + 1], scale=1.0
                    )
                else:
                    nc.vector.tensor_scalar_add(qreg, pch, bqbr[:, t : t + 1])
                if pend is not None:
                    q_tail(*pend)
                pend = (t, n)
            # scalar reaches these after chunk n's evac/exp work: paced issue
            nc.scalar.sem_inc(pace_sem, 1)
            if n + 2 < NS:
                for k in (6, 7):
                    xtc_dma(nc.scalar, n + 2, k)
            kv_prefetch(n)
        q_tail(*pend)

        # Q pool finalizers: gq_all = gq_true/64 (br contribution removed).
        for t in range(T):
            tsl = slice(t * NS, (t + 1) * NS)
            stot = small_pool.tile([P, 1], F32, name="stot", tag="stot")
            nc.vector.reduce_sum(stot, pe_q[:, tsl], axis=AX)
            rec = small_pool.tile([P, 1], F32, name="rec", tag="rec")
            nc.vector.reciprocal(rec, stot)
            if POOL_MODE == "split":
                nc.vector.tensor_scalar_mul(rec, rec, INV_W8)
            graw = small_pool.tile([P, 1], F32, name="graw", tag="graw")
            nc.vector.reduce_sum(graw, pq_q[:, tsl], axis=AX)
            tmp = small_pool.tile([P, 1], F32, name="gtmp", tag="gtmp")
            nc.vector.tensor_tensor(tmp, graw, rec, ALU.mult)
            nc.vector.tensor_tensor(gq_all[:, t : t + 1], tmp, br64[:], ALU.subtract)

        # ---- K phase: t-outer, chunk-PAIR pipelined gated logits ----
        # One 2-bank GEMM psum pair per two s-chunks; a single evacuation,
        # exp+denominator, and fused pooled-sum op each cover the whole pair,
        # halving DVE/ACT instruction counts so both stay under TensorE's
        # 1080ns/slot fp8 pace.
        NP2 = NS // 2

        def k_tail(t, np_, pt, eff):
            pl = psl.tile([P, 2, NC_], F32, name="pl", tag="pl")
            nc.tensor.matmul(pl[:, 0, :], eff[:], pt[:, 0, :], start=True, stop=True)
            nc.tensor.matmul(pl[:, 1, :], eff[:], pt[:, 1, :], start=True, stop=True)
            col = slice(t * NP2 + np_, t * NP2 + np_ + 1)
            e = e_pool.tile([P, 2, NC_], BF16, name="e", tag="e")
            nc.scalar.activation(
                e, pl, AF.Exp, bias=0.0, scale=1.0, accum_out=pe_k[:, col]
            )
            pool_sum(e, pt, pq_k[:, col])

        def k_final(t):
            tsl = slice(t * NP2, (t + 1) * NP2)
            stot = small_pool.tile([P, 1], F32, name="stot", tag="stot")
            nc.vector.reduce_sum(stot, pe_k[:, tsl], axis=AX)
            rec = small_pool.tile([P, 1], F32, name="rec", tag="rec")
            nc.vector.reciprocal(rec, stot)
            if POOL_MODE == "split":
                nc.vector.tensor_scalar_mul(rec, rec, INV_W8)
            graw = small_pool.tile([P, 1], F32, name="graw", tag="graw")
            nc.vector.reduce_sum(graw, pq_k[:, tsl], axis=AX)
            tmp = small_pool.tile([P, 1], F32, name="gtmp", tag="gtmp")
            nc.vector.tensor_tensor(tmp, graw, rec, ALU.mult)
            nc.vector.tensor_tensor(
                gk_all[:, t : t + 1], tmp, gq_all[:, t : t + 1], ALU.mult
            )

        kpend = None
        for t in range(T):
            eff = eff_pool.tile([P, P], BF16, name="effkl", tag="eff")
            nc.vector.tensor_scalar_mul(eff[:], wklrep[:], gq_all[:, t : t + 1])
            for np_ in range(NP2):
                pg = psg.tile([P, 2, NC_], F32, name="pg", tag="pg")
                for i in (0, 1):
                    n = 2 * np_ + i
                    for kk in range(0, T, 2):
                        nc.tensor.matmul(
                            pg[:, i, :],
                            wk[:, kk : kk + 2, t * P : (t + 1) * P],
                            xt8[:, kk : kk + 2, n * NC_ : (n + 1) * NC_],
                            start=(kk == 0), stop=(kk == T - 2), perf_mode=DR,
                        )
                pt = chk_pool.tile([P, 2, NC_], BF16, name="chk", tag="chk")
                if np_ % 2 == 0:
                    nc.scalar.activation(
                        pt, pg, AF.Identity, bias=bk64[:, t : t + 1], scale=1.0
                    )
                else:
                    nc.vector.tensor_scalar_add(pt, pg, bk64[:, t : t + 1])
                if kpend is not None:
                    k_tail(*kpend)
                    if kpend[1] == NP2 - 1:
                        k_final(kpend[0])
                kpend = (t, np_, pt, eff)
        k_tail(*kpend)
        k_final(T - 1)

        # ---- V phase: chunk-pair rt matmuls + residual add + store ----
        def v_tail(t, np_, ut, eff):
            pl = psl.tile([P, 2, NC_], F32, name="pl", tag="pl")
            for i in (0, 1):
                n = 2 * np_ + i
                nc.tensor.matmul(
                    pl[:, i, :], eff[:], ut[:, i, :], start=True, stop=True
                )
                stg = stg_pool.tile([P, NC_], OUT_DT, name="stg", tag="stg")
                nc.vector.tensor_tensor(stg, pl[:, i, :], qreg_of(t, n), ALU.add)
                osl = slice(t * P, (t + 1) * P)
                csl = slice(n * NC_, (n + 1) * NC_)
                if t < T - 1:
                    dma_eng = nc.sync if n % 2 == 0 else nc.gpsimd
                    dma_eng.dma_start(out_ext.ap()[osl, csl], stg)
                else:
                    # final tile: split each store across the two DMA queues
                    # that have no compute left so the last transfers drain
                    # on multiple rings
                    h = NC_ // 2
                    nc.sync.dma_start(
                        out_ext.ap()[osl, n * NC_ : n * NC_ + h], stg[:, :h]
                    )
                    nc.gpsimd.dma_start(
                        out_ext.ap()[osl, n * NC_ + h : (n + 1) * NC_], stg[:, h:]
                    )

        vpend = None
        for t in range(T):
            eff = eff_pool.tile([P, P], BF16, name="effrt", tag="eff")
            nc.vector.tensor_scalar_mul(eff[:], wrr[:], gk_all[:, t : t + 1])
            for np_ in range(NP2):
                pg = psg.tile([P, 2, NC_], F32, name="pg", tag="pg")
                for i in (0, 1):
                    n = 2 * np_ + i
                    for kk in range(0, T, 2):
                        nc.tensor.matmul(
                            pg[:, i, :],
                            wv[:, kk : kk + 2, t * P : (t + 1) * P],
                            xt8[:, kk : kk + 2, n * NC_ : (n + 1) * NC_],
                            start=(kk == 0), stop=(kk == T - 2), perf_mode=DR,
                        )
                ut = chk_pool.tile([P, 2, NC_], BF16, name="chk", tag="chk")
                nc.scalar.activation(
                    ut, pg, AF.Identity, bias=bv64[:, t : t + 1], scale=1.0
                )
                if vpend is not None:
                    v_tail(*vpend)
                vpend = (t, np_, ut, eff)
        v_tail(*vpend)

    nc.compile()
    return nc


def _prep_shared(inputs):
    """Host-side prep of the replicated (weight) arrays."""
    sc = 0.125  # 1/sqrt(HD)

    def rep_logit(w):
        m = np.zeros((P, P), dtype=np.float32)
        ws = w.astype(np.float32) * sc
        m[:HD, :HD] = ws[:, None]          # rows d 0..63 -> head-0 columns
        m[HD:, HD:] = ws[:, None]          # rows d 64..127 -> head-1 columns
        return m.astype(BF)

    def bias_pp(b):
        return np.ascontiguousarray(b.astype(np.float32).reshape(T, P).T)

    wrr = np.zeros((P, P), dtype=np.float32)
    wr = inputs["Wr"].astype(np.float32)
    wrr[:HD, :HD] = wr
    wrr[HD:, HD:] = wr

    br_col = np.tile(inputs["br"].astype(np.float32), 2).reshape(P, 1)

    # [k*P+p, t*P+j] -> [p][t][k][j]
    wqt = np.ascontiguousarray(
        inputs["Wq"].astype(BF).reshape(T, P, T, P).transpose(1, 2, 0, 3)
    ).reshape(P, T * T * P)

    return {
        "wq": wqt,
        "wk": np.ascontiguousarray(
            (inputs["Wk"].astype(np.float32) * W8SCALE).astype(F8)
        ),
        "wv": np.ascontiguousarray(
            (inputs["Wv"].astype(np.float32) * W8SCALE).astype(F8)
        ),
        "bqbr": bias_pp(inputs["bq"]) + br_col,
        "bk64": bias_pp(inputs["bk"]) * np.float32(W8SCALE),
        "bv64": bias_pp(inputs["bv"]) * np.float32(W8SCALE),
        "br64": np.ascontiguousarray(br_col * np.float32(INV_W8)),
        "wqlrep": rep_logit(inputs["wql"]),
        "wklrep": rep_logit(inputs["wkl"]),
        "wrr": wrr.astype(BF),
    }


def _get_nc():
    if "nc" not in _CACHE:
        _CACHE["nc"] = _build()
    return _CACHE["nc"]


def _run(inputs, trace=False):
    nc = _get_nc()
    shared = _prep_shared(inputs)
    X = inputs["X"]
    in_maps = []
    for b in range(N_CORES):
        m = dict(shared)
        xtb = np.ascontiguousarray(X[b].T)
        m["xt"] = xtb.astype(BF)
        m["xt8"] = xtb.astype(F8)
        in_maps.append(m)
    if trace:
        _install_profile_hook()
    res = run_bass_kernel_spmd(nc, in_maps, list(range(N_CORES)), trace=trace)
    out = np.empty((B, S, D), dtype=np.float32)
    for b in range(N_CORES):
        out[b] = np.asarray(res.results[b]["out"]).astype(np.float32).T
    return out, res


def _install_profile_hook():
    import antenv

    if "antenv.axon_hooks" not in sys.modules:
        mod = types.ModuleType("antenv.axon_hooks")
        mod._hook = None
        mod.set_axon_ntff_profile_hook = lambda h: setattr(mod, "_hook", h)
        mod.get_axon_ntff_profile_hook = lambda: mod._hook
        sys.modules["antenv.axon_hooks"] = mod
        antenv.axon_hooks = mod
    hooks = sys.modules["antenv.axon_hooks"]
    if hooks.get_axon_ntff_profile_hook() is None:
        from trn_agent_boot.trn_boot import _ntff_profile_via_ctypes

        hooks.set_axon_ntff_profile_hook(
            _ntff_profile_via_ctypes("/opt/axon/libaxon_pjrt.so")
        )
    import concourse.bass_utils as bass_utils

    bass_utils.upload_artifacts = lambda tmpdir: f"local:{tmpdir}"


def kernel(**inputs) -> np.ndarray:
    out, _ = _run(inputs, trace=False)
    return out
